# revision 1
# baseline (speedup 1.0000x reference)
"""Self-contained Trainium2 Bass kernel for the GAT layer problem
nn_GATLayer_57062935494774 (V=50000, E=800000, IN=256, OUT=128, alpha=0.2).

kernel(**inputs) takes the full unsharded inputs (x, W, a, edge_index),
distributes across 8 NeuronCores, and returns the full (V, 128) output.

Distribution: output rows are sorted by degree and grouped into 128-row
tiles (degree-homogeneous); tiles are dealt round-robin to the 8 cores so
per-core edge counts balance. Each core runs an identical SPMD program:

  Phase 1: every core computes the full augmented node table
    T[v] = [Wh[v] (128) | s_dst[v] | 1.0 | pad]  (132 fp32 = 528 B rows)
    from xT/W/a (x transposed on host).
  Phase 2: per (row-tile, neighbor-slot): indirect-DMA gather of 128 table
    rows (one per partition, keyed by the edge's source node); s_src comes
    from a per-tile matmul over host-permuted xTrows; e = lrelu(s_src +
    s_dst); phi = exp(e) * pad-mask; aggregation accumulates
    PSUM += diag(phi_d) @ G_d[:, :130] (features + ones column giving the
    softmax denominator); out = elu(num/den). Softmax max-subtraction is
    skipped (scores are bounded, exp is exact in fp32 range).
"""

import numpy as np

P = 128
TW = 132          # table row width (fp32)
NRHS = 130        # matmul rhs width: 128 feats + s_dst + ones
ALPHA = 0.2
NCORES = 8
TPC = 49          # row tiles per core (8*49*128 = 50176 >= 50000)


# ------------------------------------------------------------------ fixes

def _install_legalizer():
    """This walrus build allows only ONE sync wait per instruction; Tile
    emits several. Split extra waits into standalone EventSemaphore
    instructions on the same engine (same blocking semantics)."""
    import orjson
    import concourse.bass2jax as b2j
    import concourse.bass_utils as bu

    if getattr(b2j, "_legalizer_installed", False):
        return

    def legalize(bir):
        d = orjson.loads(bir)
        ctr = 0
        changed = False
        for fn in d.get("functions", []):
            for blk in fn.get("blocks", []):
                new = []
                for inst in blk.get("instructions", []):
                    si = inst.get("sync_info")
                    waits = si.get("on_wait", []) if si else []
                    if len(waits) > 1:
                        changed = True
                        for w in waits[:-1]:
                            ctr += 1
                            new.append({
                                "debug": inst.get("debug", 0),
                                "engine": inst["engine"],
                                "ins": [], "outs": [],
                                "name": f"lgw{ctr}_{inst.get('name', '')}"[:64],
                                "opcode": "EventSemaphore",
                                "sync_info": {"on_update": [], "on_wait": [w]},
                            })
                        si["on_wait"] = [waits[-1]]
                    new.append(inst)
                blk["instructions"] = new
        return orjson.dumps(d) if changed else bir

    orig = bu.compile_bir_kernel

    def wrapped(bir_json, tmpdir, neff_name="file.neff"):
        if isinstance(bir_json, str):
            bir_json = bir_json.encode()
        return orig(legalize(bir_json), tmpdir, neff_name=neff_name)

    b2j.compile_bir_kernel = wrapped
    b2j._legalizer_installed = True


# ------------------------------------------------------------------ host prep

def _host_prep(x, W, a, edge_index):
    import concourse.mybir as mybir  # noqa: F401  (ensures concourse importable)

    V, IN = x.shape
    row = np.asarray(edge_index[0]).astype(np.int64)
    col = np.asarray(edge_index[1]).astype(np.int64)

    ntiles = NCORES * TPC
    nslots = ntiles * P
    vt_tiles = NCORES * ((V + NCORES * P - 1) // (NCORES * P))
    vpad = vt_tiles * P

    deg = np.bincount(row, minlength=V)
    degp = np.concatenate([deg, np.zeros(nslots - V, np.int64)])
    order = np.argsort(-degp, kind="stable")
    tile_rows = order.reshape(ntiles, P)
    tile_maxdeg = np.where(tile_rows < V, deg[np.minimum(tile_rows, V - 1)], 0).max(1)

    gidx = np.arange(ntiles).reshape(TPC, NCORES)
    F_sched = np.maximum(tile_maxdeg[gidx].max(1), 1).astype(np.int64)
    nslots_e = int(F_sched.sum())

    eorder = np.argsort(row, kind="stable")
    col_s = col[eorder]
    rstart = np.searchsorted(row[eorder], np.arange(V))
    rend = np.searchsorted(row[eorder], np.arange(V), side="right")

    xTf = np.asarray(x, np.float32).T
    slot_off = np.concatenate([[0], np.cumsum(F_sched)])
    in_maps, row_perm = [], np.empty((NCORES, TPC * P), np.int64)
    xT = np.zeros((IN, vpad), np.float32)
    xT[:, :V] = xTf
    a_np = np.asarray(a, np.float32)
    a2 = np.ascontiguousarray(np.stack([a_np[:128], a_np[128:]], axis=1))
    W_np = np.ascontiguousarray(np.asarray(W, np.float32))

    for c in range(NCORES):
        offs = np.zeros((P, nslots_e), np.int32)
        mask = np.zeros((P, nslots_e), np.float32)
        rows_of_core = np.empty(TPC * P, np.int64)
        for j in range(TPC):
            rl = tile_rows[j * NCORES + c]
            rows_of_core[j * P:(j + 1) * P] = rl
            o = slot_off[j]
            for p in range(P):
                r = rl[p]
                if r >= V:
                    continue
                n = rend[r] - rstart[r]
                offs[p, o:o + n] = col_s[rstart[r]:rstart[r] + n]
                mask[p, o:o + n] = 1.0
        row_perm[c] = rows_of_core
        xr = np.zeros((IN, TPC * P), np.float32)
        real = rows_of_core < V
        xr[:, real] = xTf[:, rows_of_core[real]]
        in_maps.append({
            "xT": xT, "W": W_np, "a2": a2,
            "xTrows": np.ascontiguousarray(xr),
            "offs": offs, "mask": mask,
        })

    meta = dict(F_sched=F_sched.tolist(), vt_tiles=vt_tiles,
                row_perm=row_perm, V=V)
    return in_maps, meta


# ------------------------------------------------------------------ kernel build

def _build_kernel(F_sched, vt_tiles, xb=8, ob=4):
    import concourse.bass as bass
    import concourse.mybir as mybir
    import concourse.tile as tile

    F32 = mybir.dt.float32
    I32 = mybir.dt.int32
    AF = mybir.ActivationFunctionType
    OP = mybir.AluOpType

    vpad = vt_tiles * P
    nrows = TPC * P
    nslots_e = int(sum(F_sched))
    slot_off = [0]
    for f in F_sched:
        slot_off.append(slot_off[-1] + f)

    nc = bass.Bass("TRN2")
    xT = nc.dram_tensor("xT", [256, vpad], F32, kind="ExternalInput")
    W = nc.dram_tensor("W", [256, P], F32, kind="ExternalInput")
    a2 = nc.dram_tensor("a2", [P, 2], F32, kind="ExternalInput")
    xTrows = nc.dram_tensor("xTrows", [256, nrows], F32, kind="ExternalInput")
    offs = nc.dram_tensor("offs", [P, nslots_e], I32, kind="ExternalInput")
    mask = nc.dram_tensor("mask", [P, nslots_e], F32, kind="ExternalInput")
    out = nc.dram_tensor("out", [nrows, P], F32, kind="ExternalOutput")
    T_dram = nc.dram_tensor("T_tab", [vpad, TW], F32, kind="Internal")

    with tile.TileContext(nc) as tc:
        with (
            tc.tile_pool(name="const", bufs=1) as cpool,
            tc.tile_pool(name="xt", bufs=3) as xtpool,
            tc.tile_pool(name="tb", bufs=3) as tbpool,
            tc.tile_pool(name="meta", bufs=1) as mpool,
            tc.tile_pool(name="g", bufs=8) as gpool,
            tc.tile_pool(name="sm", bufs=3) as smpool,
            tc.tile_pool(name="dg", bufs=4) as dgpool,
            tc.tile_pool(name="ob", bufs=2) as opool,
            tc.tile_pool(name="ps", bufs=2, space="PSUM") as pspool,
            tc.tile_pool(name="ps2", bufs=1, space="PSUM") as ps2pool,
            tc.tile_pool(name="pss", bufs=2, space="PSUM") as psspool,
        ):
            iota_i = cpool.tile([P, P], I32)
            nc.gpsimd.iota(iota_i[:], pattern=[[1, P]], base=0, channel_multiplier=0)
            iota_f = cpool.tile([P, P], F32)
            nc.vector.tensor_copy(iota_f[:], iota_i[:])
            iotap_i = cpool.tile([P, 1], I32)
            nc.gpsimd.iota(iotap_i[:], pattern=[[1, 1]], base=0, channel_multiplier=1)
            iotap_f = cpool.tile([P, 1], F32)
            nc.vector.tensor_copy(iotap_f[:], iotap_i[:])
            ident = cpool.tile([P, P], F32)
            nc.vector.tensor_scalar(out=ident[:], in0=iota_f[:], scalar1=iotap_f[:],
                                    scalar2=None, op0=OP.is_equal)

            offs_t = mpool.tile([P, nslots_e], I32)
            nc.sync.dma_start(offs_t[:], offs[:])
            mask_t = mpool.tile([P, nslots_e], F32)
            nc.sync.dma_start(mask_t[:], mask[:])
            xtr_t = mpool.tile([P, 2 * nrows], F32)
            nc.sync.dma_start(xtr_t[:, 0:nrows], xTrows[0:P, :])
            nc.sync.dma_start(xtr_t[:, nrows:2 * nrows], xTrows[P:2 * P, :])

            a2_t = cpool.tile([P, 2], F32)
            nc.sync.dma_start(a2_t[:], a2[:])
            rhs_big, wtil_sb = [], []
            for c in range(2):
                wc = cpool.tile([P, P], F32, tag=f"wc{c}")
                nc.sync.dma_start(wc[:], W[c * P:(c + 1) * P, :])
                wt_ps = ps2pool.tile([P, P], F32)
                nc.tensor.transpose(out=wt_ps[:], in_=wc[:], identity=ident[:])
                wt_sb = cpool.tile([P, P], F32, tag=f"wt{c}")
                nc.vector.tensor_copy(wt_sb[:], wt_ps[:])
                wtil_ps = ps2pool.tile([P, 2], F32)
                nc.tensor.matmul(wtil_ps[:], lhsT=wt_sb[:], rhs=a2_t[:],
                                 start=True, stop=True)
                wt2 = cpool.tile([P, 2], F32, tag=f"wt2{c}")
                nc.vector.tensor_copy(wt2[:], wtil_ps[:])
                wtil_sb.append(wt2)
                rb = cpool.tile([P, NRHS - 1], F32, tag=f"rb{c}")
                nc.vector.tensor_copy(rb[:, 0:P], wc[:])
                nc.vector.tensor_copy(rb[:, P:P + 1], wt2[:, 1:2])
                rhs_big.append(rb)

            for b in range(vt_tiles // xb):
                xt0 = xtpool.tile([P, xb * P], F32, tag="xt0")
                nc.sync.dma_start(xt0[:], xT[0:P, b * xb * P:(b + 1) * xb * P])
                xt1 = xtpool.tile([P, xb * P], F32, tag="xt1")
                nc.sync.dma_start(xt1[:], xT[P:2 * P, b * xb * P:(b + 1) * xb * P])
                tb = tbpool.tile([P, xb * TW], F32, tag="tb")
                nc.vector.memset(tb[:, P + 1::TW], 1.0)
                nc.vector.memset(tb[:, P + 2::TW], 0.0)
                nc.vector.memset(tb[:, P + 3::TW], 0.0)
                for q in range(xb):
                    ps = pspool.tile([P, NRHS - 1], F32, tag="p1")
                    nc.tensor.matmul(ps[:], lhsT=xt0[:, q * P:(q + 1) * P],
                                     rhs=rhs_big[0][:], start=True, stop=False)
                    nc.tensor.matmul(ps[:], lhsT=xt1[:, q * P:(q + 1) * P],
                                     rhs=rhs_big[1][:], start=False, stop=True)
                    nc.vector.tensor_copy(tb[:, q * TW:q * TW + P], ps[:, 0:P])
                    nc.scalar.activation(tb[:, q * TW + P:q * TW + P + 1],
                                         ps[:, P:P + 1], AF.Copy)
                dst = bass.AP(T_dram, (b * xb * P) * TW,
                              [[TW, P], [P * TW, xb], [1, TW]])
                nc.sync.dma_start(dst, tb[:])

            tc.strict_bb_all_engine_barrier()

            outb = None
            for j in range(TPC):
                Fj = int(F_sched[j])
                o0 = slot_off[j]
                ps_s = psspool.tile([P, 1], F32, tag="pss")
                nc.tensor.matmul(ps_s[:], lhsT=xtr_t[:, j * P:(j + 1) * P],
                                 rhs=wtil_sb[0][:, 0:1], start=True, stop=False)
                nc.tensor.matmul(ps_s[:],
                                 lhsT=xtr_t[:, nrows + j * P:nrows + (j + 1) * P],
                                 rhs=wtil_sb[1][:, 0:1], start=False, stop=True)
                sv = smpool.tile([P, 1], F32, tag="sv")
                nc.scalar.activation(sv[:], ps_s[:], AF.Copy)

                gts = []
                sd = smpool.tile([P, max(Fj, 1)], F32, tag="sd")
                for dslot in range(Fj):
                    gt = gpool.tile([P, TW], F32, tag=f"gt{dslot % 8}")
                    nc.gpsimd.indirect_dma_start(
                        out=gt[:], out_offset=None, in_=T_dram[:],
                        in_offset=bass.IndirectOffsetOnAxis(
                            ap=offs_t[:, o0 + dslot:o0 + dslot + 1], axis=0),
                    )
                    gts.append(gt)
                    nc.vector.tensor_copy(sd[:, dslot:dslot + 1], gt[:, P:P + 1])

                u = smpool.tile([P, max(Fj, 1)], F32, tag="u")
                nc.vector.tensor_scalar(out=u[:], in0=sd[:], scalar1=sv[:],
                                        scalar2=None, op0=OP.add)
                ua = smpool.tile([P, max(Fj, 1)], F32, tag="ua")
                nc.vector.tensor_scalar(out=ua[:], in0=u[:], scalar1=ALPHA,
                                        scalar2=None, op0=OP.mult)
                lr = smpool.tile([P, max(Fj, 1)], F32, tag="lr")
                nc.vector.tensor_tensor(out=lr[:], in0=u[:], in1=ua[:], op=OP.max)
                phi = smpool.tile([P, max(Fj, 1)], F32, tag="phi")
                nc.scalar.activation(phi[:], lr[:], AF.Exp)
                phm = smpool.tile([P, max(Fj, 1)], F32, tag="phm")
                nc.vector.tensor_tensor(out=phm[:], in0=phi[:],
                                        in1=mask_t[:, o0:o0 + Fj], op=OP.mult)

                ps = pspool.tile([P, NRHS], F32, tag="p2")
                for dslot in range(Fj):
                    dg = dgpool.tile([P, P], F32, tag=f"dg{dslot % 4}")
                    nc.vector.tensor_scalar(out=dg[:], in0=ident[:],
                                            scalar1=phm[:, dslot:dslot + 1],
                                            scalar2=None, op0=OP.mult)
                    nc.tensor.matmul(ps[:], lhsT=dg[:], rhs=gts[dslot][:, 0:NRHS],
                                     start=(dslot == 0), stop=(dslot == Fj - 1))

                if j % ob == 0:
                    outb = opool.tile([P, ob * P], F32, tag="outb")
                oc = (j % ob) * P
                den = smpool.tile([P, 1], F32, tag="den")
                nc.vector.tensor_scalar(out=den[:], in0=ps[:, P + 1:P + 2],
                                        scalar1=1e-30, scalar2=None, op0=OP.max)
                rden = smpool.tile([P, 1], F32, tag="rden")
                nc.vector.reciprocal(rden[:], den[:])
                res = outb[:, oc:oc + P]
                nc.vector.tensor_scalar(out=res, in0=ps[:, 0:P],
                                        scalar1=rden[:], scalar2=None, op0=OP.mult)
                t1 = smpool.tile([P, P], F32, tag="t1")
                nc.vector.tensor_scalar(out=t1[:], in0=res, scalar1=0.0,
                                        scalar2=-1.0, op0=OP.max, op1=OP.add)
                t2 = smpool.tile([P, P], F32, tag="t2")
                nc.vector.tensor_scalar(out=t2[:], in0=res, scalar1=0.0,
                                        scalar2=None, op0=OP.min)
                t3 = smpool.tile([P, P], F32, tag="t3")
                nc.scalar.activation(t3[:], t2[:], AF.Exp)
                nc.vector.tensor_tensor(out=res, in0=t1[:], in1=t3[:], op=OP.add)
                if j % ob == ob - 1 or j == TPC - 1:
                    n = j % ob + 1
                    j0 = j - n + 1
                    dst = bass.AP(out, (j0 * P) * P, [[P, P], [P * P, n], [1, P]])
                    nc.sync.dma_start(dst, outb[:, :n * P])
    return nc


# ------------------------------------------------------------------ entry

def kernel(x, W, a, edge_index):
    _install_legalizer()
    from concourse.bass_utils import run_bass_kernel_spmd

    x = np.asarray(x)
    in_maps, meta = _host_prep(x, W, a, edge_index)
    nc = _build_kernel(meta["F_sched"], meta["vt_tiles"])
    res = run_bass_kernel_spmd(nc, in_maps, core_ids=list(range(NCORES)))

    V = meta["V"]
    row_perm = meta["row_perm"]
    full = np.zeros((V, P), np.float32)
    for c, r in enumerate(res.results):
        rp = row_perm[c]
        valid = rp < V
        full[rp[valid]] = r["out"][valid]
    return full



# revision 3
# speedup vs baseline: 1.0686x; 1.0686x over previous
"""Self-contained Trainium2 Bass kernel for the GAT layer problem
nn_GATLayer_57062935494774 (V=50000, E=800000, IN=256, OUT=128, alpha=0.2).

kernel(**inputs) takes the full unsharded inputs (x, W, a, edge_index),
distributes across 8 NeuronCores, and returns the full (V, 128) output.

Distribution: output rows are sorted by degree and grouped into 128-row
tiles (degree-homogeneous); tiles are dealt round-robin to the 8 cores so
per-core edge counts balance. Each core runs an identical SPMD program:

  Phase 1: every core computes the full augmented node table in bf16
    T[v] = [Wh[v] (128 bf16) | s_dst[v] (fp32 bit-punned into 2 bf16
    slots) | pad]  (136 bf16 = 272 B rows)
    from xT (bf16) / W (bf16) / wtil (= W @ [a_dst | a_src], bf16).
  Phase 2: per row-tile: one [128,1]-offset indirect gather per neighbor
    slot into a contiguous per-tile buffer; s_dst extracted via fp32
    bitcast; s_src from a per-tile matmul over host-permuted x rows;
    e = lrelu(s_src + s_dst); phi = exp(e) on the scalar engine, whose
    accum_out directly yields the softmax denominator. Padding slots
    point at a sentinel row whose crafted s_dst ~ -1e9 makes phi exactly
    0 (no mask needed). Aggregation: sg = phi * gathered features (one
    broadcast DVE op per tile), then PSUM += ident^T @ sg_d per slot.
    out = elu(num/den).
"""

import numpy as np

P = 128
TW = 136          # table row width in bf16 elems (272 B, 16B-aligned)
ALPHA = 0.2
NCORES = 8
TPC = 49          # row tiles per core (8*49*128 = 50176 >= 50000)
XB = 8            # node tiles per phase-1 block
OB = 4            # output tiles per write


# ------------------------------------------------------------------ fixes

def _install_legalizer():
    """This walrus build allows only ONE sync wait per instruction; Tile
    emits several. Split extra waits into standalone EventSemaphore
    instructions on the same engine (same blocking semantics)."""
    import orjson
    import concourse.bass2jax as b2j
    import concourse.bass_utils as bu

    if getattr(b2j, "_legalizer_installed", False):
        return

    def legalize(bir):
        d = orjson.loads(bir)
        ctr = 0
        changed = False
        for fn in d.get("functions", []):
            for blk in fn.get("blocks", []):
                new = []
                for inst in blk.get("instructions", []):
                    si = inst.get("sync_info")
                    waits = si.get("on_wait", []) if si else []
                    if len(waits) > 1:
                        changed = True
                        for w in waits[:-1]:
                            ctr += 1
                            new.append({
                                "debug": inst.get("debug", 0),
                                "engine": inst["engine"],
                                "ins": [], "outs": [],
                                "name": f"lgw{ctr}_{inst.get('name', '')}"[:64],
                                "opcode": "EventSemaphore",
                                "sync_info": {"on_update": [], "on_wait": [w]},
                            })
                        si["on_wait"] = [waits[-1]]
                    new.append(inst)
                blk["instructions"] = new
        return orjson.dumps(d) if changed else bir

    orig = bu.compile_bir_kernel

    def wrapped(bir_json, tmpdir, neff_name="file.neff"):
        if isinstance(bir_json, str):
            bir_json = bir_json.encode()
        return orig(legalize(bir_json), tmpdir, neff_name=neff_name)

    b2j.compile_bir_kernel = wrapped
    b2j._legalizer_installed = True


# ------------------------------------------------------------------ host prep

def _host_prep(x, W, a, edge_index):
    import ml_dtypes

    bf16 = ml_dtypes.bfloat16
    V, IN = x.shape
    row = np.asarray(edge_index[0]).astype(np.int64)
    col = np.asarray(edge_index[1]).astype(np.int64)

    ntiles = NCORES * TPC
    nslots = ntiles * P
    vt_tiles = NCORES * ((V + NCORES * P - 1) // (NCORES * P))
    vpad = vt_tiles * P
    assert vpad == nslots
    sent = vpad - 1                       # sentinel row id for padding slots

    deg = np.bincount(row, minlength=V)
    degp = np.concatenate([deg, np.zeros(nslots - V, np.int64)])
    order = np.argsort(-degp, kind="stable")
    tile_rows = order.reshape(ntiles, P)
    tile_maxdeg = np.where(tile_rows < V, deg[np.minimum(tile_rows, V - 1)], 0).max(1)

    gidx = np.arange(ntiles).reshape(TPC, NCORES)
    F_sched = np.maximum(tile_maxdeg[gidx].max(1), 1).astype(np.int64)

    eorder = np.argsort(row, kind="stable")
    col_s = col[eorder]
    rstart = np.searchsorted(row[eorder], np.arange(V))
    rend = np.searchsorted(row[eorder], np.arange(V), side="right")

    slot_off = np.concatenate([[0], np.cumsum(F_sched)])
    stot = int(slot_off[-1])

    # sentinel column of x: crafted so its s_dst (and s_src) ~ -1e9
    W64 = np.asarray(W, np.float64)
    a64 = np.asarray(a, np.float64)
    vs = W64 @ a64[:128]
    vd = W64 @ a64[128:]
    G = np.array([[vd @ vd, vs @ vd], [vd @ vs, vs @ vs]])
    c = np.linalg.solve(G, np.array([-1e9, -1e9]))
    x_sent = c[0] * vd + c[1] * vs

    xT = np.zeros((IN, vpad), np.float32)
    xT[:, :V] = np.asarray(x, np.float32).T
    xT[:, sent] = x_sent.astype(np.float32)
    xT16 = xT.astype(bf16)
    xTf = xT  # fp32 copy for xTrows build
    W16 = np.ascontiguousarray(np.asarray(W, np.float32)).astype(bf16)
    # wtil cols: [W @ a_dst, W @ a_src]
    wtil = np.stack([vd, vs], axis=1).astype(np.float32).astype(bf16)
    wtil = np.ascontiguousarray(wtil)

    in_maps, row_perm = [], np.empty((NCORES, TPC * P), np.int64)
    for cid in range(NCORES):
        offs = np.full((P, stot), sent, np.int32)
        rows_of_core = np.empty(TPC * P, np.int64)
        for j in range(TPC):
            rl = tile_rows[j * NCORES + cid]
            rows_of_core[j * P:(j + 1) * P] = rl
            o = int(slot_off[j])
            for p in range(P):
                r = rl[p]
                if r >= V:
                    continue
                n = rend[r] - rstart[r]
                offs[p, o:o + n] = col_s[rstart[r]:rstart[r] + n]
        row_perm[cid] = rows_of_core
        xr = np.zeros((IN, TPC * P), np.float32)
        real = rows_of_core < vpad
        xr[:, real] = xTf[:, rows_of_core[real]]
        in_maps.append({"xT": xT16, "W": W16, "wtil": wtil, "offs": offs,
                        "xTrows": np.ascontiguousarray(xr.astype(bf16))})

    meta = dict(F_sched=F_sched.tolist(), vt_tiles=vt_tiles,
                row_perm=row_perm, V=V)
    return in_maps, meta


# ------------------------------------------------------------------ kernel build

def _build_kernel(F_sched, vt_tiles):
    import concourse.bass as bass
    import concourse.mybir as mybir
    import concourse.tile as tile

    F32 = mybir.dt.float32
    BF16 = mybir.dt.bfloat16
    I32 = mybir.dt.int32
    AF = mybir.ActivationFunctionType
    OP = mybir.AluOpType

    vpad = vt_tiles * P
    nrows = TPC * P
    slot_off = [0]
    for f in F_sched:
        slot_off.append(slot_off[-1] + int(f))
    stot = slot_off[-1]

    nc = bass.Bass("TRN2")
    xT = nc.dram_tensor("xT", [256, vpad], BF16, kind="ExternalInput")
    W = nc.dram_tensor("W", [256, P], BF16, kind="ExternalInput")
    wtil = nc.dram_tensor("wtil", [256, 2], BF16, kind="ExternalInput")
    offs = nc.dram_tensor("offs", [P, stot], I32, kind="ExternalInput")
    xTrows = nc.dram_tensor("xTrows", [256, nrows], BF16, kind="ExternalInput")
    out = nc.dram_tensor("out", [nrows, P], F32, kind="ExternalOutput")
    T_dram = nc.dram_tensor("T_tab", [vpad, TW], BF16, kind="Internal")

    with tile.TileContext(nc) as tc:
        with (
            tc.tile_pool(name="const", bufs=1) as cpool,
            tc.tile_pool(name="xt", bufs=3) as xtpool,
            tc.tile_pool(name="tb", bufs=3) as tbpool,
            tc.tile_pool(name="meta", bufs=1) as mpool,
            tc.tile_pool(name="g", bufs=2) as gpool,
            tc.tile_pool(name="sg", bufs=2) as sgpool,
            tc.tile_pool(name="sm", bufs=3) as smpool,
            tc.tile_pool(name="ob", bufs=2) as opool,
            tc.tile_pool(name="ps", bufs=2, space="PSUM") as pspool,
            tc.tile_pool(name="ps2", bufs=2, space="PSUM") as ps2pool,
            tc.tile_pool(name="pss", bufs=2, space="PSUM") as psspool,
        ):
            # identity matrix in bf16 (for the scatter-accumulate matmuls)
            iota_i = cpool.tile([P, P], I32)
            nc.gpsimd.iota(iota_i[:], pattern=[[1, P]], base=0, channel_multiplier=0)
            iota_f = cpool.tile([P, P], F32)
            nc.vector.tensor_copy(iota_f[:], iota_i[:])
            iotap_i = cpool.tile([P, 1], I32)
            nc.gpsimd.iota(iotap_i[:], pattern=[[1, 1]], base=0, channel_multiplier=1)
            iotap_f = cpool.tile([P, 1], F32)
            nc.vector.tensor_copy(iotap_f[:], iotap_i[:])
            ident = cpool.tile([P, P], BF16)
            nc.vector.tensor_scalar(out=ident[:], in0=iota_f[:], scalar1=iotap_f[:],
                                    scalar2=None, op0=OP.is_equal)

            offs_t = mpool.tile([P, stot], I32)
            nc.sync.dma_start(offs_t[:], offs[:])
            xtr_t = mpool.tile([P, 2 * nrows], BF16)
            nc.sync.dma_start(xtr_t[:, 0:nrows], xTrows[0:P, :])
            nc.sync.dma_start(xtr_t[:, nrows:2 * nrows], xTrows[P:2 * P, :])
            wsrc = cpool.tile([P, 2], BF16)
            nc.sync.dma_start(wsrc[:, 0:1], wtil[0:P, 1:2])
            nc.sync.dma_start(wsrc[:, 1:2], wtil[P:2 * P, 1:2])

            # rhs for phase 1: [W block | wtil_dst block]
            rhs_big = []
            for ci in range(2):
                rb = cpool.tile([P, P + 1], BF16, tag=f"rb{ci}")
                nc.sync.dma_start(rb[:, 0:P], W[ci * P:(ci + 1) * P, :])
                nc.sync.dma_start(rb[:, P:P + 1], wtil[ci * P:(ci + 1) * P, 0:1])
                rhs_big.append(rb)

            # ---------------- phase 1: build the node table ----------------
            for b in range(vt_tiles // XB):
                xt0 = xtpool.tile([P, XB * P], BF16, tag="xt0")
                nc.sync.dma_start(xt0[:], xT[0:P, b * XB * P:(b + 1) * XB * P])
                xt1 = xtpool.tile([P, XB * P], BF16, tag="xt1")
                nc.sync.dma_start(xt1[:], xT[P:2 * P, b * XB * P:(b + 1) * XB * P])
                tb = tbpool.tile([P, XB, TW], BF16, tag="tb")
                for q in range(XB):
                    ps = pspool.tile([P, P + 1], F32, tag="p1")
                    nc.tensor.matmul(ps[:], lhsT=xt0[:, q * P:(q + 1) * P],
                                     rhs=rhs_big[0][:], start=True, stop=False)
                    nc.tensor.matmul(ps[:], lhsT=xt1[:, q * P:(q + 1) * P],
                                     rhs=rhs_big[1][:], start=False, stop=True)
                    # features fp32 -> bf16 (scalar engine)
                    nc.scalar.activation(tb[:, q, 0:P], ps[:, 0:P], AF.Copy)
                    # s_dst: raw fp32 bits punned into 2 bf16 cols (DVE)
                    nc.vector.tensor_copy(tb[:, q, P:P + 2].bitcast(F32),
                                          ps[:, P:P + 1])
                dst = bass.AP(T_dram, (b * XB * P) * TW,
                              [[TW, P], [P * TW, XB], [1, TW]])
                nc.sync.dma_start(dst, tb[:])

            tc.strict_bb_all_engine_barrier()

            # ---------------- phase 2: gather + softmax + aggregate --------
            outb = None
            for j in range(TPC):
                Fj = int(F_sched[j])
                o = slot_off[j]
                g = gpool.tile([P, Fj, TW], BF16, tag="g")
                for s in range(Fj):
                    nc.gpsimd.indirect_dma_start(
                        out=g[:, s, :], out_offset=None, in_=T_dram[:],
                        in_offset=bass.IndirectOffsetOnAxis(
                            ap=offs_t[:, o + s:o + s + 1], axis=0),
                    )
                # s_src for this tile's rows (exact: from permuted x rows)
                ps_s = psspool.tile([P, 1], F32, tag="pss")
                nc.tensor.matmul(ps_s[:], lhsT=xtr_t[:, j * P:(j + 1) * P],
                                 rhs=wsrc[:, 0:1], start=True, stop=False)
                nc.tensor.matmul(ps_s[:],
                                 lhsT=xtr_t[:, nrows + j * P:nrows + (j + 1) * P],
                                 rhs=wsrc[:, 1:2], start=False, stop=True)
                sv = smpool.tile([P, 1], F32, tag="sv")
                nc.scalar.activation(sv[:], ps_s[:], AF.Copy)

                # s_dst: punned fp32 in table cols 128..129
                sd = smpool.tile([P, Fj], F32, tag="sd")
                nc.vector.tensor_copy(sd[:], g[:, :, P:P + 2].bitcast(F32))
                u = smpool.tile([P, Fj], F32, tag="u")
                nc.vector.tensor_scalar(out=u[:], in0=sd[:], scalar1=sv[:],
                                        scalar2=None, op0=OP.add)
                lr = smpool.tile([P, Fj], F32, tag="lr")
                nc.vector.scalar_tensor_tensor(
                    out=lr[:], in0=u[:], scalar=ALPHA, in1=u[:],
                    op0=OP.mult, op1=OP.max)
                phi = smpool.tile([P, Fj], F32, tag="phi")
                den = smpool.tile([P, 1], F32, tag="den")
                nc.scalar.activation(phi[:], lr[:], AF.Exp, accum_out=den[:])
                phm = smpool.tile([P, Fj], BF16, tag="phm")
                nc.scalar.activation(phm[:], phi[:], AF.Copy)

                # sg[p, d, :] = phi[p, d] * feats[p, d, :]
                sg = sgpool.tile([P, Fj, P], BF16, tag="sg")
                nc.vector.tensor_tensor(
                    out=sg[:], in0=g[:, :, 0:P],
                    in1=phm[:].unsqueeze(2).broadcast_to((P, Fj, P)),
                    op=OP.mult)

                ps2 = ps2pool.tile([P, P], F32, tag="p2")
                for d in range(Fj):
                    nc.tensor.matmul(ps2[:], lhsT=ident[:], rhs=sg[:, d, :],
                                     start=(d == 0), stop=(d == Fj - 1))

                if j % OB == 0:
                    outb = opool.tile([P, OB * P], F32, tag="outb")
                oc = (j % OB) * P
                dg = smpool.tile([P, 1], F32, tag="dg")
                nc.vector.tensor_scalar(out=dg[:], in0=den[:], scalar1=1e-30,
                                        scalar2=None, op0=OP.max)
                rden = smpool.tile([P, 1], F32, tag="rden")
                nc.vector.reciprocal(rden[:], dg[:])
                res = outb[:, oc:oc + P]
                nc.vector.tensor_scalar(out=res, in0=ps2[:], scalar1=rden[:],
                                        scalar2=None, op0=OP.mult)
                # elu: max(x,0)-1 + exp(min(x,0))
                t1 = smpool.tile([P, P], F32, tag="t1")
                nc.vector.tensor_scalar(out=t1[:], in0=res, scalar1=0.0,
                                        scalar2=-1.0, op0=OP.max, op1=OP.add)
                t2 = smpool.tile([P, P], F32, tag="t2")
                nc.vector.tensor_scalar(out=t2[:], in0=res, scalar1=0.0,
                                        scalar2=None, op0=OP.min)
                t3 = smpool.tile([P, P], F32, tag="t3")
                nc.scalar.activation(t3[:], t2[:], AF.Exp)
                nc.vector.tensor_tensor(out=res, in0=t1[:], in1=t3[:], op=OP.add)
                if j % OB == OB - 1 or j == TPC - 1:
                    n = j % OB + 1
                    jb = j - n + 1
                    dst = bass.AP(out, (jb * P) * P, [[P, P], [P * P, n], [1, P]])
                    nc.sync.dma_start(dst, outb[:, :n * P])
    return nc


# ------------------------------------------------------------------ entry

def kernel(x, W, a, edge_index):
    _install_legalizer()
    from concourse.bass_utils import run_bass_kernel_spmd

    x = np.asarray(x)
    in_maps, meta = _host_prep(x, W, a, edge_index)
    nc = _build_kernel(meta["F_sched"], meta["vt_tiles"])
    res = run_bass_kernel_spmd(nc, in_maps, core_ids=list(range(NCORES)))

    V = meta["V"]
    row_perm = meta["row_perm"]
    full = np.zeros((V, P), np.float32)
    for c, r in enumerate(res.results):
        rp = row_perm[c]
        valid = rp < V
        full[rp[valid]] = r["out"][valid]
    return full


# revision 6
# speedup vs baseline: 1.0802x; 1.0109x over previous
"""Self-contained Trainium2 Bass kernel for the GAT layer problem
nn_GATLayer_57062935494774 (V=50000, E=800000, IN=256, OUT=128, alpha=0.2).

kernel(**inputs) takes the full unsharded inputs (x, W, a, edge_index),
distributes across 8 NeuronCores, and returns the full (V, 128) output.

Distribution: output rows are sorted by degree and grouped into 128-row
tiles (degree-homogeneous); tiles are dealt round-robin to the 8 cores so
per-core edge counts balance. Each core runs an identical SPMD program:

  Phase 1: every core computes the full augmented node table in bf16
    T[v] = [Wh[v] (128 bf16) | s_dst[v] (fp32 bit-punned into 2 bf16
    slots) | pad]  (136 bf16 = 272 B rows)
    from xT (bf16) / W (bf16) / wtil (= W @ [a_dst | a_src], bf16).
  Phase 2: per row-tile: one [128,1]-offset indirect gather per neighbor
    slot into a contiguous per-tile buffer; s_dst extracted via fp32
    bitcast; s_src from a per-tile matmul over host-permuted x rows;
    e = lrelu(s_src + s_dst); phi = exp(e) on the scalar engine, whose
    accum_out directly yields the softmax denominator. Padding slots
    point at a sentinel row whose crafted s_dst ~ -1e9 makes phi exactly
    0 (no mask needed). Aggregation: sg = phi * gathered features (one
    broadcast DVE op per tile), then PSUM += ident^T @ sg_d per slot.
    out = elu(num/den).
"""

import numpy as np

P = 128
TW = 136          # table row width in bf16 elems (272 B, 16B-aligned)
ALPHA = 0.2
NCORES = 8
TPC = 49          # row tiles per core (8*49*128 = 50176 >= 50000)
XB = 8            # node tiles per phase-1 block
OB = 4            # output tiles per write


# ------------------------------------------------------------------ fixes

def _install_legalizer():
    """This walrus build allows only ONE sync wait per instruction; Tile
    emits several. Split extra waits into standalone EventSemaphore
    instructions on the same engine (same blocking semantics)."""
    import orjson
    import concourse.bass2jax as b2j
    import concourse.bass_utils as bu

    if getattr(b2j, "_legalizer_installed", False):
        return

    def legalize(bir):
        d = orjson.loads(bir)
        ctr = 0
        changed = False
        for fn in d.get("functions", []):
            for blk in fn.get("blocks", []):
                new = []
                for inst in blk.get("instructions", []):
                    si = inst.get("sync_info")
                    waits = si.get("on_wait", []) if si else []
                    if len(waits) > 1:
                        changed = True
                        for w in waits[:-1]:
                            ctr += 1
                            new.append({
                                "debug": inst.get("debug", 0),
                                "engine": inst["engine"],
                                "ins": [], "outs": [],
                                "name": f"lgw{ctr}_{inst.get('name', '')}"[:64],
                                "opcode": "EventSemaphore",
                                "sync_info": {"on_update": [], "on_wait": [w]},
                            })
                        si["on_wait"] = [waits[-1]]
                    new.append(inst)
                blk["instructions"] = new
        return orjson.dumps(d) if changed else bir

    orig = bu.compile_bir_kernel

    def wrapped(bir_json, tmpdir, neff_name="file.neff"):
        if isinstance(bir_json, str):
            bir_json = bir_json.encode()
        return orig(legalize(bir_json), tmpdir, neff_name=neff_name)

    b2j.compile_bir_kernel = wrapped
    b2j._legalizer_installed = True


# ------------------------------------------------------------------ host prep

def _host_prep(x, W, a, edge_index):
    import ml_dtypes

    bf16 = ml_dtypes.bfloat16
    V, IN = x.shape
    row = np.asarray(edge_index[0]).astype(np.int64)
    col = np.asarray(edge_index[1]).astype(np.int64)

    ntiles = NCORES * TPC
    nslots = ntiles * P
    vt_tiles = NCORES * ((V + NCORES * P - 1) // (NCORES * P))
    vpad = vt_tiles * P
    assert vpad == nslots
    sent = vpad - 1                       # sentinel row id for padding slots

    deg = np.bincount(row, minlength=V)
    degp = np.concatenate([deg, np.zeros(nslots - V, np.int64)])
    order = np.argsort(-degp, kind="stable")
    tile_rows = order.reshape(ntiles, P)
    tile_maxdeg = np.where(tile_rows < V, deg[np.minimum(tile_rows, V - 1)], 0).max(1)

    gidx = np.arange(ntiles).reshape(TPC, NCORES)
    F_sched = np.maximum(tile_maxdeg[gidx].max(1), 1).astype(np.int64)

    eorder = np.argsort(row, kind="stable")
    col_s = col[eorder]
    rstart = np.searchsorted(row[eorder], np.arange(V))
    rend = np.searchsorted(row[eorder], np.arange(V), side="right")

    slot_off = np.concatenate([[0], np.cumsum(F_sched)])
    stot = int(slot_off[-1])

    # sentinel column of x: crafted so its s_dst (and s_src) ~ -1e9
    W64 = np.asarray(W, np.float64)
    a64 = np.asarray(a, np.float64)
    vs = W64 @ a64[:128]
    vd = W64 @ a64[128:]
    G = np.array([[vd @ vd, vs @ vd], [vd @ vs, vs @ vs]])
    c = np.linalg.solve(G, np.array([-1e9, -1e9]))
    x_sent = c[0] * vd + c[1] * vs

    xT = np.zeros((IN, vpad), np.float32)
    xT[:, :V] = np.asarray(x, np.float32).T
    xT[:, sent] = x_sent.astype(np.float32)
    xT16 = xT.astype(bf16)
    xTf = xT  # fp32 copy for xTrows build
    W16 = np.ascontiguousarray(np.asarray(W, np.float32)).astype(bf16)
    # wtil cols: [W @ a_dst, W @ a_src]
    wtil = np.stack([vd, vs], axis=1).astype(np.float32).astype(bf16)
    wtil = np.ascontiguousarray(wtil)

    in_maps, row_perm = [], np.empty((NCORES, TPC * P), np.int64)
    for cid in range(NCORES):
        offs = np.full((P, stot), sent, np.int32)
        rows_of_core = np.empty(TPC * P, np.int64)
        for j in range(TPC):
            rl = tile_rows[j * NCORES + cid]
            rows_of_core[j * P:(j + 1) * P] = rl
            o = int(slot_off[j])
            for p in range(P):
                r = rl[p]
                if r >= V:
                    continue
                n = rend[r] - rstart[r]
                offs[p, o:o + n] = col_s[rstart[r]:rstart[r] + n]
        row_perm[cid] = rows_of_core
        xr = np.zeros((IN, TPC * P), np.float32)
        real = rows_of_core < vpad
        xr[:, real] = xTf[:, rows_of_core[real]]
        in_maps.append({"xT": xT16, "W": W16, "wtil": wtil, "offs": offs,
                        "xTrows": np.ascontiguousarray(xr.astype(bf16))})

    meta = dict(F_sched=F_sched.tolist(), vt_tiles=vt_tiles,
                row_perm=row_perm, V=V)
    return in_maps, meta


# ------------------------------------------------------------------ kernel build

def _build_kernel(F_sched, vt_tiles):
    import concourse.bass as bass
    import concourse.mybir as mybir
    import concourse.tile as tile

    F32 = mybir.dt.float32
    BF16 = mybir.dt.bfloat16
    I32 = mybir.dt.int32
    AF = mybir.ActivationFunctionType
    OP = mybir.AluOpType

    vpad = vt_tiles * P
    nrows = TPC * P
    slot_off = [0]
    for f in F_sched:
        slot_off.append(slot_off[-1] + int(f))
    stot = slot_off[-1]

    nc = bass.Bass("TRN2")
    xT = nc.dram_tensor("xT", [256, vpad], BF16, kind="ExternalInput")
    W = nc.dram_tensor("W", [256, P], BF16, kind="ExternalInput")
    wtil = nc.dram_tensor("wtil", [256, 2], BF16, kind="ExternalInput")
    offs = nc.dram_tensor("offs", [P, stot], I32, kind="ExternalInput")
    xTrows = nc.dram_tensor("xTrows", [256, nrows], BF16, kind="ExternalInput")
    out = nc.dram_tensor("out", [nrows, P], F32, kind="ExternalOutput")
    T_dram = nc.dram_tensor("T_tab", [vpad, TW], BF16, kind="Internal")

    with tile.TileContext(nc) as tc:
        with (
            tc.tile_pool(name="const", bufs=1) as cpool,
            tc.tile_pool(name="xt", bufs=3) as xtpool,
            tc.tile_pool(name="tb", bufs=3) as tbpool,
            tc.tile_pool(name="meta", bufs=1) as mpool,
            tc.tile_pool(name="g", bufs=3) as gpool,
            tc.tile_pool(name="sg", bufs=3) as sgpool,
            tc.tile_pool(name="sm", bufs=4) as smpool,
            tc.tile_pool(name="ob", bufs=2) as opool,
            tc.tile_pool(name="ps", bufs=2, space="PSUM") as pspool,
            tc.tile_pool(name="ps2", bufs=4, space="PSUM") as ps2pool,
            tc.tile_pool(name="pss", bufs=2, space="PSUM") as psspool,
        ):
            # identity matrix in bf16 (for the scatter-accumulate matmuls)
            iota_i = cpool.tile([P, P], I32)
            nc.gpsimd.iota(iota_i[:], pattern=[[1, P]], base=0, channel_multiplier=0)
            iota_f = cpool.tile([P, P], F32)
            nc.vector.tensor_copy(iota_f[:], iota_i[:])
            iotap_i = cpool.tile([P, 1], I32)
            nc.gpsimd.iota(iotap_i[:], pattern=[[1, 1]], base=0, channel_multiplier=1)
            iotap_f = cpool.tile([P, 1], F32)
            nc.vector.tensor_copy(iotap_f[:], iotap_i[:])
            ident = cpool.tile([P, P], BF16)
            nc.vector.tensor_scalar(out=ident[:], in0=iota_f[:], scalar1=iotap_f[:],
                                    scalar2=None, op0=OP.is_equal)

            offs_t = mpool.tile([P, stot], I32)
            nc.sync.dma_start(offs_t[:], offs[:])
            xtr_t = mpool.tile([P, 2 * nrows], BF16)
            nc.sync.dma_start(xtr_t[:, 0:nrows], xTrows[0:P, :])
            nc.sync.dma_start(xtr_t[:, nrows:2 * nrows], xTrows[P:2 * P, :])
            wsrc = cpool.tile([P, 2], BF16)
            nc.sync.dma_start(wsrc[:, 0:1], wtil[0:P, 1:2])
            nc.sync.dma_start(wsrc[:, 1:2], wtil[P:2 * P, 1:2])

            # rhs for phase 1: [W block | wtil_dst block]
            rhs_big = []
            for ci in range(2):
                rb = cpool.tile([P, P + 1], BF16, tag=f"rb{ci}")
                nc.sync.dma_start(rb[:, 0:P], W[ci * P:(ci + 1) * P, :])
                nc.sync.dma_start(rb[:, P:P + 1], wtil[ci * P:(ci + 1) * P, 0:1])
                rhs_big.append(rb)

            # ---------------- phase 1: build the node table ----------------
            for b in range(vt_tiles // XB):
                xt0 = xtpool.tile([P, XB * P], BF16, tag="xt0")
                nc.sync.dma_start(xt0[:], xT[0:P, b * XB * P:(b + 1) * XB * P])
                xt1 = xtpool.tile([P, XB * P], BF16, tag="xt1")
                nc.sync.dma_start(xt1[:], xT[P:2 * P, b * XB * P:(b + 1) * XB * P])
                tb = tbpool.tile([P, XB, TW], BF16, tag="tb")
                for q in range(XB):
                    ps = pspool.tile([P, P + 1], F32, tag="p1")
                    nc.tensor.matmul(ps[:], lhsT=xt0[:, q * P:(q + 1) * P],
                                     rhs=rhs_big[0][:], start=True, stop=False)
                    nc.tensor.matmul(ps[:], lhsT=xt1[:, q * P:(q + 1) * P],
                                     rhs=rhs_big[1][:], start=False, stop=True)
                    # features fp32 -> bf16, split across scalar + vector
                    if q % 2 == 0:
                        nc.scalar.activation(tb[:, q, 0:P], ps[:, 0:P], AF.Copy)
                    else:
                        nc.vector.tensor_copy(tb[:, q, 0:P], ps[:, 0:P])
                    # s_dst: raw fp32 bits punned into 2 bf16 cols (DVE)
                    nc.vector.tensor_copy(tb[:, q, P:P + 2].bitcast(F32),
                                          ps[:, P:P + 1])
                dst = bass.AP(T_dram, (b * XB * P) * TW,
                              [[TW, P], [P * TW, XB], [1, TW]])
                nc.sync.dma_start(dst, tb[:])

            tc.strict_bb_all_engine_barrier()

            # ---------------- phase 2: gather + softmax + aggregate --------
            outb = None
            for j in range(TPC):
                Fj = int(F_sched[j])
                o = slot_off[j]
                g = gpool.tile([P, Fj, TW], BF16, tag="g")
                for s in range(Fj):
                    nc.gpsimd.indirect_dma_start(
                        out=g[:, s, :], out_offset=None, in_=T_dram[:],
                        in_offset=bass.IndirectOffsetOnAxis(
                            ap=offs_t[:, o + s:o + s + 1], axis=0),
                    )
                # s_src for this tile's rows (exact: from permuted x rows)
                ps_s = psspool.tile([P, 1], F32, tag="pss")
                nc.tensor.matmul(ps_s[:], lhsT=xtr_t[:, j * P:(j + 1) * P],
                                 rhs=wsrc[:, 0:1], start=True, stop=False)
                nc.tensor.matmul(ps_s[:],
                                 lhsT=xtr_t[:, nrows + j * P:nrows + (j + 1) * P],
                                 rhs=wsrc[:, 1:2], start=False, stop=True)
                sv = smpool.tile([P, 1], F32, tag="sv")
                nc.scalar.activation(sv[:], ps_s[:], AF.Copy)

                # s_dst: punned fp32 in table cols 128..129
                sd = smpool.tile([P, Fj], F32, tag="sd")
                nc.vector.tensor_copy(sd[:], g[:, :, P:P + 2].bitcast(F32))
                u = smpool.tile([P, Fj], F32, tag="u")
                nc.vector.tensor_scalar(out=u[:], in0=sd[:], scalar1=sv[:],
                                        scalar2=None, op0=OP.add)
                lr = smpool.tile([P, Fj], F32, tag="lr")
                nc.vector.scalar_tensor_tensor(
                    out=lr[:], in0=u[:], scalar=ALPHA, in1=u[:],
                    op0=OP.mult, op1=OP.max)
                phi = smpool.tile([P, Fj], F32, tag="phi")
                den = smpool.tile([P, 1], F32, tag="den")
                nc.scalar.activation(phi[:], lr[:], AF.Exp, accum_out=den[:])
                phm = smpool.tile([P, Fj], BF16, tag="phm")
                nc.scalar.activation(phm[:], phi[:], AF.Copy)

                # sg[p, d, :] = phi[p, d] * feats[p, d, :]
                sg = sgpool.tile([P, Fj, P], BF16, tag="sg")
                nc.vector.tensor_tensor(
                    out=sg[:], in0=g[:, :, 0:P],
                    in1=phm[:].unsqueeze(2).broadcast_to((P, Fj, P)),
                    op=OP.mult)

                ps2 = ps2pool.tile([P, P], F32, tag="p2")
                for d in range(Fj):
                    nc.tensor.matmul(ps2[:], lhsT=ident[:], rhs=sg[:, d, :],
                                     start=(d == 0), stop=(d == Fj - 1))

                if j % OB == 0:
                    outb = opool.tile([P, OB * P], F32, tag="outb")
                oc = (j % OB) * P
                dg = smpool.tile([P, 1], F32, tag="dg")
                nc.vector.tensor_scalar(out=dg[:], in0=den[:], scalar1=1e-30,
                                        scalar2=None, op0=OP.max)
                rden = smpool.tile([P, 1], F32, tag="rden")
                nc.vector.reciprocal(rden[:], dg[:])
                res = outb[:, oc:oc + P]
                nc.vector.tensor_scalar(out=res, in0=ps2[:], scalar1=rden[:],
                                        scalar2=None, op0=OP.mult)
                # elu: max(x,0)-1 + exp(min(x,0))
                t1 = smpool.tile([P, P], F32, tag="t1")
                nc.vector.tensor_scalar(out=t1[:], in0=res, scalar1=0.0,
                                        scalar2=-1.0, op0=OP.max, op1=OP.add)
                t2 = smpool.tile([P, P], F32, tag="t2")
                nc.vector.tensor_scalar(out=t2[:], in0=res, scalar1=0.0,
                                        scalar2=None, op0=OP.min)
                t3 = smpool.tile([P, P], F32, tag="t3")
                nc.scalar.activation(t3[:], t2[:], AF.Exp)
                nc.vector.tensor_tensor(out=res, in0=t1[:], in1=t3[:], op=OP.add)
                if j % OB == OB - 1 or j == TPC - 1:
                    n = j % OB + 1
                    jb = j - n + 1
                    dst = bass.AP(out, (jb * P) * P, [[P, P], [P * P, n], [1, P]])
                    nc.sync.dma_start(dst, outb[:, :n * P])
    return nc


# ------------------------------------------------------------------ entry

def kernel(x, W, a, edge_index):
    _install_legalizer()
    from concourse.bass_utils import run_bass_kernel_spmd

    x = np.asarray(x)
    in_maps, meta = _host_prep(x, W, a, edge_index)
    nc = _build_kernel(meta["F_sched"], meta["vt_tiles"])
    res = run_bass_kernel_spmd(nc, in_maps, core_ids=list(range(NCORES)))

    V = meta["V"]
    row_perm = meta["row_perm"]
    full = np.zeros((V, P), np.float32)
    for c, r in enumerate(res.results):
        rp = row_perm[c]
        valid = rp < V
        full[rp[valid]] = r["out"][valid]
    return full
